# revision 1
# baseline (speedup 1.0000x reference)
"""CenterLoss on 8 Trainium2 NeuronCores.

reference math:
    distances = ||x_i||^2 + ||c_j||^2 - 2 x_i.c_j   (full [B, C])
    out = mean_i distances[i, labels[i]]

Key simplification: only each sample's own-class center row is needed, so
instead of a [4096, 7001] distance matrix we gather centers[labels] (an
indirect DMA) and compute mean_i ||x_i - c_{l_i}||^2.

Sharding: data-parallel over the batch. Each of the 8 cores gets 512
samples (x shard + label shard) and a full replicated copy of `centers`
(stays in HBM; only the 512 gathered rows are ever read). Each core
reduces its shard to a single partial scalar (sum of its selected
distances / 4096); the host sums the 8 partial scalars.

Per-core layout: sample s of the shard maps to (partition p, block t) with
s = p*4 + t, so both the x load and the label load are single contiguous
DMAs ([128, 2048] and [128, 4]).
"""

import numpy as np

import bass_rust
import concourse.bass as bass
import concourse.tile as tile
from concourse import mybir
from concourse.bass_utils import run_bass_kernel_spmd

B = 4096          # global batch
C = 7001          # num classes
D = 512           # embed dim
N_CORES = 8
BS = B // N_CORES  # 512 samples per core
P = 128            # SBUF partitions
NT = BS // P       # 4 sample-blocks per partition

_NC_CACHE = {}


def _split_multiwait(nc):
    """The walrus build here encodes at most ONE sync-wait per instruction
    ("Too many sync wait commands" codegen error otherwise).  Tile attaches
    every required wait to the consuming instruction, so hoist all but the
    last wait into standalone EventSemaphore instructions on the same
    engine — semantically identical (the sequencer processes them in
    order), and exactly how raw-bass wait_ge encodes waits."""
    for fn in nc.m.functions:
        for bb in fn.blocks:
            new = []
            changed = False
            for ins in bb.instructions:
                si = ins.sync_info
                if si is not None and len(si.on_wait) > 1:
                    waits = list(si.on_wait)
                    for j, w in enumerate(waits[:-1]):
                        new.append(mybir.InstEventSemaphore(
                            name=f"{ins.name}-prewait{j}",
                            opcode="EventSemaphore",
                            engine=ins.engine,
                            sync_info=bass_rust.SyncInfo(on_wait=[w], on_update=[]),
                        ))
                    ins.sync_info = bass_rust.SyncInfo(
                        on_wait=[waits[-1]], on_update=list(si.on_update))
                    changed = True
                new.append(ins)
            if changed:
                bb.instructions = new
    return nc


def _trim_tail_barrier(nc):
    """Drop the second all-engine barrier butterfly after the end-of-kernel
    semaphore sweep ("doing this twice just to be safe" in bass finalize).
    Butterfly #1 and the sweep stay; the barrier sems are neutral after #1,
    and the NEXT execution's main-block barrier already keeps every engine
    from touching swept sems before Pool finishes sweeping.  Saves ~2 us of
    counted tail (the measured window ends at last engine activity)."""
    bb = nc.m.functions[0].blocks[-1]
    insts = list(bb.instructions)
    isa_idx = max(i for i, ins in enumerate(insts)
                  if type(ins).__name__ == 'InstISA')
    keep, dropped = insts[:isa_idx + 1], 0
    for ins in insts[isa_idx + 1:]:
        tn = type(ins).__name__
        if tn in ('InstDrain', 'InstEventSemaphore'):
            dropped += 1
            continue
        keep.append(ins)
    assert dropped == 11, dropped
    bb.instructions = keep
    return nc


def _build_bass():
    nc = bass.Bass()

    x = nc.dram_tensor("x", [BS, D], mybir.dt.float32, kind="ExternalInput")
    centers = nc.dram_tensor("centers", [C, D], mybir.dt.float32, kind="ExternalInput")
    labels = nc.dram_tensor("labels", [BS, 1], mybir.dt.int32, kind="ExternalInput")
    out = nc.dram_tensor("out", [1, 1], mybir.dt.float32, kind="ExternalOutput")

    # sample s = p*NT + t lives at partition p, free block t
    x_view = x[:].rearrange("(p t) d -> p (t d)", t=NT)        # [128, 2048]
    lab_view = labels[:].rearrange("(p t) u -> p (t u)", t=NT)  # [128, 4]

    with tile.TileContext(nc) as tc:
        with (
            tc.tile_pool(name="big", bufs=1) as big,
            tc.tile_pool(name="small", bufs=1) as small,
            tc.tile_pool(name="psum", bufs=1, space="PSUM") as psum,
        ):
            xt = big.tile([P, NT * D], mybir.dt.float32)
            ct = big.tile([P, NT * D], mybir.dt.float32)
            diff = big.tile([P, NT * D], mybir.dt.bfloat16)
            sq = big.tile([P, NT * D], mybir.dt.bfloat16)
            labt = small.tile([P, NT], mybir.dt.int32)
            dist4 = small.tile([P, NT], mybir.dt.float32)
            dist = small.tile([P, 1], mybir.dt.float32)
            ones = small.tile([P, 1], mybir.dt.float32)
            res = small.tile([1, 1], mybir.dt.float32)
            acc = psum.tile([1, 1], mybir.dt.float32)

            # labels first, on the same SP HWDGE ring as the x load: HWDGE
            # rings are FIFO per issuing engine, so the 2 KB label transfer
            # completes (~1 us) before the 1 MB x transfer starts, letting
            # the gathers overlap with the x stream instead of queuing
            # behind it.
            nc.sync.dma_start(out=labt[:], in_=lab_view)
            nc.sync.dma_start(out=xt[:], in_=x_view)

            # per 512-col block: gather centers[labels] (gpsimd SWDGE),
            # diff on DVE, square + row-sum fused on ACT.  DVE and ACT
            # pipeline behind the gather stream.
            for t in range(NT):
                blk = slice(t * D, (t + 1) * D)
                nc.gpsimd.indirect_dma_start(
                    out=ct[:, blk],
                    out_offset=None,
                    in_=centers[:],
                    in_offset=bass.IndirectOffsetOnAxis(ap=labt[:, t:t + 1], axis=0),
                )
                nc.vector.tensor_sub(diff[:, blk], xt[:, blk], ct[:, blk])
                nc.scalar.activation(
                    out=sq[:, blk],
                    in_=diff[:, blk],
                    func=mybir.ActivationFunctionType.Square,
                    accum_out=dist4[:, t:t + 1],
                )

            # dist[p] = sum_t dist4[p, t]; partition-reduce via PE into a
            # single scalar so the output store is one dense 4 B descriptor
            # (a [128,1] store is 128 4-byte descriptors whose completion
            # costs ~10 us on the tail).
            # Split the partition-reduce: blocks 0..NT-2 reduce + matmul into
            # PSUM while the last gather/square is still in flight; only one
            # accumulating matmul remains on the critical path after the
            # last square.
            nc.vector.reduce_sum(out=dist[:], in_=dist4[:, 0:NT - 1],
                                 axis=mybir.AxisListType.X)
            nc.vector.memset(ones[:], 1.0 / B)
            nc.tensor.matmul(out=acc[:], lhsT=dist[:], rhs=ones[:],
                             start=True, stop=False)
            nc.tensor.matmul(out=acc[:], lhsT=dist4[:, NT - 1:NT], rhs=ones[:],
                             start=False, stop=True)
            nc.vector.tensor_copy(out=res[:], in_=acc[:])
            nc.sync.dma_start(out=out[:], in_=res[:])

    _split_multiwait(nc)
    _trim_tail_barrier(nc)
    return nc


def _get_nc():
    if "nc" not in _NC_CACHE:
        _NC_CACHE["nc"] = _build_bass()
    return _NC_CACHE["nc"]


def kernel(**inputs: np.ndarray) -> np.ndarray:
    x = np.ascontiguousarray(np.asarray(inputs["x"], dtype=np.float32))
    centers = np.ascontiguousarray(np.asarray(inputs["centers"], dtype=np.float32))
    labels = np.asarray(inputs["labels"]).astype(np.int32).reshape(B, 1)

    nc = _get_nc()
    in_maps = [
        {
            "x": x[c * BS:(c + 1) * BS],
            "centers": centers,
            "labels": np.ascontiguousarray(labels[c * BS:(c + 1) * BS]),
        }
        for c in range(N_CORES)
    ]
    res = run_bass_kernel_spmd(nc, in_maps, core_ids=list(range(N_CORES)))
    # unshard: each core returns (sum of its selected squared distances)/B;
    # the global mean is the sum of the 8 partials.
    total = np.float32(0.0)
    for r in res.results:
        total += r["out"][0, 0]
    return np.array(total, dtype=np.float32)



# revision 2
# speedup vs baseline: 1.0778x; 1.0778x over previous
"""CenterLoss on 8 Trainium2 NeuronCores.

reference math:
    distances = ||x_i||^2 + ||c_j||^2 - 2 x_i.c_j   (full [B, C])
    out = mean_i distances[i, labels[i]]

Key simplification: only each sample's own-class center row is needed, so
instead of a [4096, 7001] distance matrix we gather centers[labels] (an
indirect DMA) and compute mean_i ||x_i - c_{l_i}||^2.

Sharding: data-parallel over the batch. Each of the 8 cores gets 512
samples (x shard + label shard) and a full replicated copy of `centers`
(stays in HBM; only the 512 gathered rows are ever read). Each core
reduces its shard to a single partial scalar (sum of its selected
distances / 4096); the host sums the 8 partial scalars.

Per-core layout: sample s of the shard maps to (partition p, block t) with
s = p*4 + t, so both the x load and the label load are single contiguous
DMAs ([128, 2048] and [128, 4]).

v2 changes (from the 22.3us baseline trace):
  - x load issued on SP HWDGE as the very first post-prologue instruction
    (hoisted into IR block 0, ahead of the removed kernel-entry barrier);
    labels go on the ACT HWDGE ring so neither queues behind the other.
  - The bass kernel-entry all-engine barrier is deleted: every user
    instruction already carries exact semaphore deps, and cross-execution
    ordering is host-enforced (nrt waits for all engines' end-of-kernel
    notify before ringing the next doorbell).
  - 2 multi-offset indirect gathers ([128, 2, 512] each) instead of 4
    single-offset ones: halves the serialized 994ns-per-DMA SWDGE
    descriptor-generation cost on the Pool/Q7 engine.
  - 4 compute blocks (DVE sub -> ACT square+accum) pipelined per gather
    half, each feeding an accumulating PE matmul (start on block 0, stop
    on block 3), so no DVE reduce and only one tiny matmul sits on the
    critical path after the last square.
  - The end-of-kernel SP drain kept only its out-store completion wait;
    its other 9 waits are implied transitively by the instructions that
    already ran (each DMA sem was waited on by a consumer that a later
    instruction in the chain depends on).
"""

import numpy as np

import bass_rust
import concourse.bass as bass
import concourse.tile as tile
from concourse import mybir
from concourse.bass_utils import run_bass_kernel_spmd

B = 4096          # global batch
C = 7001          # num classes
D = 512           # embed dim
N_CORES = 8
BS = B // N_CORES  # 512 samples per core
P = 128            # SBUF partitions
NT = BS // P       # 4 sample-blocks per partition
NG = 2             # indirect gathers (NT // NG offsets per gather)
GT = NT // NG      # sample-blocks per gather
NB = 4             # compute blocks (sub + square-accum + matmul each)
CB = NT * D // NB  # columns per compute block

_NC_CACHE = {}


def _split_multiwait(nc):
    """The walrus build here encodes at most ONE sync-wait per instruction
    ("Too many sync wait commands" codegen error otherwise).  Tile attaches
    every required wait to the consuming instruction, so hoist all but the
    last wait into standalone EventSemaphore instructions on the same
    engine — semantically identical (the sequencer processes them in
    order), and exactly how raw-bass wait_ge encodes waits."""
    for fn in nc.m.functions:
        for bb in fn.blocks:
            new = []
            changed = False
            for ins in bb.instructions:
                si = ins.sync_info
                if si is not None and len(si.on_wait) > 1:
                    waits = list(si.on_wait)
                    for j, w in enumerate(waits[:-1]):
                        new.append(mybir.InstEventSemaphore(
                            name=f"{ins.name}-prewait{j}",
                            opcode="EventSemaphore",
                            engine=ins.engine,
                            sync_info=bass_rust.SyncInfo(on_wait=[w], on_update=[]),
                        ))
                    ins.sync_info = bass_rust.SyncInfo(
                        on_wait=[waits[-1]], on_update=list(si.on_update))
                    changed = True
                new.append(ins)
            if changed:
                bb.instructions = new
    return nc


def _trim_tail_barrier(nc):
    """Drop the second all-engine barrier butterfly after the end-of-kernel
    semaphore sweep ("doing this twice just to be safe" in bass finalize).
    Butterfly #1 and the sweep stay; the barrier sems are neutral after #1,
    and the NEXT execution's doorbell already keeps every engine from
    touching swept sems before Pool finishes sweeping.  Saves ~2 us of
    counted tail (the measured window ends at last engine activity)."""
    bb = nc.m.functions[0].blocks[-1]
    insts = list(bb.instructions)
    isa_idx = max(i for i, ins in enumerate(insts)
                  if type(ins).__name__ == 'InstISA')
    keep, dropped = insts[:isa_idx + 1], 0
    for ins in insts[isa_idx + 1:]:
        tn = type(ins).__name__
        if tn in ('InstDrain', 'InstEventSemaphore'):
            dropped += 1
            continue
        keep.append(ins)
    assert dropped == 11, dropped
    bb.instructions = keep
    return nc


def _sem_names(si):
    names = []
    if si is not None:
        for w in si.on_wait:
            names.append(w.ant_name or "")
        for u in si.on_update:
            names.append(u.ant_name or "")
    return names


def _drop_entry_barrier(nc):
    """Remove the kernel-entry all-engine barrier from block 0.  Every user
    instruction carries its own semaphore deps (Tile inserted them), the
    engine preambles/memsets have no cross-engine consumers before the tail
    barrier, and cross-execution ordering is enforced by the host (nrt only
    rings the next doorbell after all engines notified completion).  The
    barrier sems (gather/release) stay untouched at 0, so the end-of-kernel
    barrier still works."""
    bb = nc.m.functions[0].blocks[0]
    keep = []
    dropped = 0
    for ins in bb.instructions:
        tn = type(ins).__name__
        if tn in ('InstDrain', 'InstEventSemaphore') and any(
                'barrier_' in n for n in _sem_names(ins.sync_info)):
            dropped += 1
            continue
        keep.append(ins)
    # 4 engines x (Drain + EventSemaphore) + Pool's gather/release pair
    assert dropped == 10, dropped
    bb.instructions = keep
    return nc


def _hoist_input_dmas(nc):
    """Move the x and labels input loads (no waits — inputs are host-written
    before the doorbell) from block 1 to the top of block 0, ahead of each
    engine's register-init moves.  With the entry barrier gone, SP/ACT issue
    them immediately after the walrus prologue."""
    fn = nc.m.functions[0]
    b0, b1 = fn.blocks[0], fn.blocks[1]
    hoist = []
    rest = []
    for ins in b1.instructions:
        if (type(ins).__name__ == 'InstDMACopy'
                and ins.engine in (mybir.EngineType.SP,
                                   mybir.EngineType.Activation)
                and (ins.sync_info is None or not ins.sync_info.on_wait)
                and len(hoist) < 2):
            hoist.append(ins)
        else:
            rest.append(ins)
    assert len(hoist) == 2, len(hoist)
    b1.instructions = rest
    # keep the leading InstCall (function entry) first
    b0.instructions = b0.instructions[:1] + hoist + b0.instructions[1:]
    return nc


def _slim_tail_drain(nc):
    """The end-of-kernel SP drain waits on every sem Tile saw outstanding.
    All but the out-store completion are implied transitively: gathers
    waited the labels sem, subs waited the x+gather sems, ACT waited DVE,
    PE waited ACT, the copy waited PE, and the out store waited the copy.
    Keep only the out-store DMA sem so the tail doesn't serialize 9 extra
    EventSemaphore hops after the result is already in flight."""
    fn = nc.m.functions[0]
    # the out store is the last SP DMACopy in block 1; its update sem is the
    # completion sem the drain must keep.
    out_sem = None
    for ins in fn.blocks[1].instructions:
        if (type(ins).__name__ == 'InstDMACopy'
                and ins.engine == mybir.EngineType.SP):
            out_sem = ins.sync_info.on_update[0].id
    assert out_sem is not None
    bb = fn.blocks[-1]
    for ins in bb.instructions:
        if (type(ins).__name__ == 'InstDrain'
                and ins.engine == mybir.EngineType.SP
                and ins.sync_info is not None
                and len(ins.sync_info.on_wait) > 1):
            waits = [w for w in ins.sync_info.on_wait if w.id == out_sem]
            assert len(waits) == 1
            ins.sync_info = bass_rust.SyncInfo(
                on_wait=waits, on_update=list(ins.sync_info.on_update))
            return nc
    raise AssertionError("tail SP drain not found")


def _build_bass():
    nc = bass.Bass()

    x = nc.dram_tensor("x", [BS, D], mybir.dt.float32, kind="ExternalInput")
    centers = nc.dram_tensor("centers", [C, D], mybir.dt.float32, kind="ExternalInput")
    labels = nc.dram_tensor("labels", [BS, 1], mybir.dt.int32, kind="ExternalInput")
    out = nc.dram_tensor("out", [1, 1], mybir.dt.float32, kind="ExternalOutput")

    # sample s = p*NT + t lives at partition p, free block t
    x_view = x[:].rearrange("(p t) d -> p (t d)", t=NT)        # [128, 2048]
    lab_view = labels[:].rearrange("(p t) u -> p (t u)", t=NT)  # [128, 4]

    with tile.TileContext(nc) as tc:
        with (
            tc.tile_pool(name="big", bufs=1) as big,
            tc.tile_pool(name="small", bufs=1) as small,
            tc.tile_pool(name="psum", bufs=1, space="PSUM") as psum,
        ):
            xt = big.tile([P, NT * D], mybir.dt.float32)
            ct = big.tile([P, NT * D], mybir.dt.float32)
            diff = big.tile([P, NT * D], mybir.dt.bfloat16)
            sq = big.tile([P, NT * D], mybir.dt.bfloat16)
            labt = small.tile([P, NT], mybir.dt.int32)
            dist4 = small.tile([P, NB], mybir.dt.float32)
            ones = small.tile([P, 1], mybir.dt.float32)
            res = small.tile([1, 1], mybir.dt.float32)
            acc = psum.tile([1, 1], mybir.dt.float32)

            # x on the SP HWDGE ring, labels on the ACT HWDGE ring: separate
            # FIFOs, so the 1 MB x stream starts immediately and the 2 KB
            # label load lands in parallel (~2.2 us end-to-end each).  Both
            # get hoisted to block 0 by _hoist_input_dmas.
            nc.sync.dma_start(out=xt[:], in_=x_view)
            nc.scalar.dma_start(out=labt[:], in_=lab_view)
            nc.vector.memset(ones[:], 1.0 / B)

            # NG multi-offset gathers: gather g covers sample-blocks
            # [g*GT, (g+1)*GT) — offsets labt[:, g*GT:(g+1)*GT] drive one
            # descriptor per (partition, block), 2 KB each.
            for g in range(NG):
                blk = slice(g * GT * D, (g + 1) * GT * D)
                nc.gpsimd.indirect_dma_start(
                    out=ct[:, blk],
                    out_offset=None,
                    in_=centers[:],
                    in_offset=bass.IndirectOffsetOnAxis(
                        ap=labt[:, g * GT:(g + 1) * GT], axis=0),
                )

            # NB compute blocks: diff on DVE, square + row-sum fused on ACT,
            # then an accumulating PE matmul folds each block's per-partition
            # sums into the single PSUM scalar (ones = 1/B).
            for b in range(NB):
                blk = slice(b * CB, (b + 1) * CB)
                nc.vector.tensor_sub(diff[:, blk], xt[:, blk], ct[:, blk])
                nc.scalar.activation(
                    out=sq[:, blk],
                    in_=diff[:, blk],
                    func=mybir.ActivationFunctionType.Square,
                    accum_out=dist4[:, b:b + 1],
                )
                nc.tensor.matmul(out=acc[:], lhsT=dist4[:, b:b + 1],
                                 rhs=ones[:],
                                 start=(b == 0), stop=(b == NB - 1))

            nc.vector.tensor_copy(out=res[:], in_=acc[:])
            nc.sync.dma_start(out=out[:], in_=res[:])

    _drop_entry_barrier(nc)
    _hoist_input_dmas(nc)
    _slim_tail_drain(nc)
    _split_multiwait(nc)
    _trim_tail_barrier(nc)
    return nc


def _get_nc():
    if "nc" not in _NC_CACHE:
        _NC_CACHE["nc"] = _build_bass()
    return _NC_CACHE["nc"]


def kernel(**inputs: np.ndarray) -> np.ndarray:
    x = np.ascontiguousarray(np.asarray(inputs["x"], dtype=np.float32))
    centers = np.ascontiguousarray(np.asarray(inputs["centers"], dtype=np.float32))
    labels = np.asarray(inputs["labels"]).astype(np.int32).reshape(B, 1)

    nc = _get_nc()
    in_maps = [
        {
            "x": x[c * BS:(c + 1) * BS],
            "centers": centers,
            "labels": np.ascontiguousarray(labels[c * BS:(c + 1) * BS]),
        }
        for c in range(N_CORES)
    ]
    res = run_bass_kernel_spmd(nc, in_maps, core_ids=list(range(N_CORES)))
    # unshard: each core returns (sum of its selected squared distances)/B;
    # the global mean is the sum of the 8 partials.
    total = np.float32(0.0)
    for r in res.results:
        total += r["out"][0, 0]
    return np.array(total, dtype=np.float32)
